# revision 28
# baseline (speedup 1.0000x reference)
"""CapsNet-CIFAR100 forward pass on 8 Trainium2 NeuronCores.

Data-parallel over batch (8 images/core). Conv stem + primary caps as
matmuls (f32r / bf16 fast-path streaming); dynamic routing with every
26M-element u_hat pass produced and consumed at bf16:

  pass 0: s0 = sum_i u_hat via dense-u matmuls (u_hat never formed),
          wr streamed bf16.
  pass 1/2: software-pipelined groups of G=8 chunks.  Per iteration gi:
      exp(gi-1) [ACT] -> softmax tail of gi-1 (z, 1/z, zsel, p16) [DVE+Pool]
      -> i-sum s-matmuls of gi-2 [PE] -> produce group gi (wr DMA, block-diag
      bf16 matmuls -> PSUM halves, ACT exits to bf16 SBUF, d-major [(d,o)])
      -> b-logit head of gi (dm = uh*v, fold tree in place) [DVE+Pool].
  The 1/z softmax normalizer is folded into the i-sum matmul's selector
  operand (zsel), so c never materializes.

Capsule chunking: chunk cb in 0..127, H=cb//64, r=cb%64; the chunk's 16
capsules are (cp in {0,1}, oh in 0..7) with ch = 128H+64cp+r, i = ch*8+oh,
vector dim k = ow. Partition index within chunk: p = cp*64 + oh*8 + ow.
conv2 runs "transposed" (output partitions = (b%2, oh, ow), free = co) so
the u gather is 32 contiguous [64,64] SBUF DMAs.
"""

from contextlib import ExitStack

import numpy as np
import ml_dtypes
import concourse.bass as bass
import concourse.mybir as mybir
import concourse.tile as tile
from concourse import bacc
from concourse import bass_utils

F32 = mybir.dt.float32
F32R = mybir.dt.float32r
F16 = mybir.dt.float16
AF = mybir.ActivationFunctionType
ALU = mybir.AluOpType
AX = mybir.AxisListType

N_CORES = 8
B = 8            # batch per core
G = 8            # routing chunks per consumer group
OSPL = 84        # o-split: [0:OSPL] on DVE, [OSPL:100] on GpSimd
EPS = 1e-8

_CACHE = {}


def _build():
    nc = bacc.Bacc("TRN2", target_bir_lowering=False, debug=False,
                   num_devices=N_CORES)

    xd = nc.dram_tensor("x_sh", [B, 3, 32, 32], F32, kind="ExternalInput").ap()
    w1d = nc.dram_tensor("w1t", [9, 27, 256], F16, kind="ExternalInput").ap()
    cbd = nc.dram_tensor("cb", [256, 1], F32, kind="ExternalInput").ap()
    w2d = nc.dram_tensor("w2t", [2, 128, 81, 256], F16, kind="ExternalInput").ap()
    pbd = nc.dram_tensor("pb", [1, 256], F32, kind="ExternalInput").ap()
    wrd = nc.dram_tensor("wr", [128, 128, 1600], F16, kind="ExternalInput").ap()
    mkd = nc.dram_tensor("mask", [128, 16, 8], F16, kind="ExternalInput").ap()
    seld = nc.dram_tensor("sel", [128, 8], F16, kind="ExternalInput").ap()
    gd = nc.dram_tensor("gmat", [128, 16], F16, kind="ExternalInput").ap()
    fdram = nc.dram_tensor("fscratch", [4, 16, 256], F32, kind="Internal").ap()
    vdram = nc.dram_tensor("vscratch", [8, 1600], F16, kind="Internal").ap()
    vout = nc.dram_tensor("v_out", [B, 100, 16], F32, kind="ExternalOutput").ap()

    with tile.TileContext(nc) as tc:
        with ExitStack() as stack:
            cpool = stack.enter_context(tc.tile_pool(name="consts", bufs=1))
            rpool = stack.enter_context(tc.tile_pool(name="rconsts", bufs=1))

            # ---------- shared constants ----------
            w1sb = cpool.tile([27, 9, 256], F16, name="w1sb")
            nc.sync.dma_start(out=w1sb, in_=w1d.rearrange("k c o -> c k o"))
            cbsb = cpool.tile([128, 2, 1], F32, name="cbsb")
            nc.sync.dma_start(out=cbsb, in_=cbd.rearrange("(t p) one -> p t one", p=128))
            pbrep = cpool.tile([128, 256], F32, name="pbrep")
            nc.sync.dma_start(
                out=pbrep,
                in_=bass.AP(tensor=pbd.tensor, offset=0, ap=[[0, 128], [1, 256]]))
            epssb = cpool.tile([128, 1], F32, name="epssb")
            nc.vector.memset(epssb, EPS)
            gsb = cpool.tile([128, 16], F16, name="gsb")
            nc.sync.dma_start(out=gsb, in_=gd)
            masksb = cpool.tile([128, 16, 8], F16, name="masksb")
            nc.sync.dma_start(out=masksb, in_=mkd)
            sel16 = cpool.tile([128, 8], F16, name="sel16")
            nc.sync.dma_start(out=sel16, in_=seld)

            # routing-persistent tiles
            ubig = rpool.tile([128, 128, B], F16, name="ubig")
            vrep = rpool.tile([128, 16, 100], F16, name="vrep")
            b1sb = rpool.tile([128, 128, 100], F16, name="b1sb")
            v2sb = rpool.tile([8, 100, 16], F32, name="v2sb")

            # ---------- conv stages (scoped pools; freed before routing) ----
            with tc.tile_pool(name="work", bufs=2) as wpool, \
                 tc.tile_pool(name="acts", bufs=1) as apool:
                # stage A: conv1 [B,3,32,32] -> h [256, B, 24, 24]
                # xsb[(ci,kw), b, r, w] = x[b, ci, r, w+kw]; contraction over
                # (ci,kw)=27, accumulate over kh in PSUM.
                with tc.tile_pool(name="hpool", bufs=1) as hpool:
                    hsb = [hpool.tile([128, B, 24, 24], F16, name="hsb",
                                      tag=f"h{c}") for c in range(2)]
                    with tc.tile_pool(name="imp", bufs=1) as impool, \
                         tc.tile_pool(name="psc", bufs=2, space="PSUM") as pscpool:
                        xsf = impool.tile([27, B, 32, 24], F32, name="xsf")
                        for ci in range(3):
                            for kw in range(9):
                                src = bass.AP(
                                    tensor=xd.tensor,
                                    offset=ci * 1024 + kw,
                                    ap=[[3072, B], [32, 32], [1, 24]],
                                )
                                nc.sync.dma_start(
                                    out=xsf[ci * 9 + kw:ci * 9 + kw + 1], in_=src)
                        xsb = impool.tile([27, B, 32, 24], F16, name="xsb")
                        nc.vector.tensor_copy(xsb, xsf)

                        for oc in range(2):
                            for b in range(B):
                                for hh in range(2):
                                    ph = pscpool.tile([128, 288], F32, name="ph",
                                                      tag="pconv")
                                    for kh in range(9):
                                        nc.tensor.matmul(
                                            ph,
                                            lhsT=w1sb[:, kh, oc * 128:(oc + 1) * 128],
                                            rhs=xsb[:, b, kh + hh * 12:
                                                    kh + hh * 12 + 12, :].rearrange(
                                                        "c h w -> c (h w)"),
                                            start=(kh == 0), stop=(kh == 8),
                                        )
                                    nc.scalar.activation(
                                        hsb[oc][:, b, hh * 12:(hh + 1) * 12, :].rearrange(
                                            "p h w -> p (h w)"),
                                        ph, AF.Relu, bias=cbsb[:, oc],
                                    )

                    # stage B: conv2 (transposed) -> p2sb
                    p2sb = [apool.tile([128, 256], F32, name="p2sb",
                                       tag=f"p2sb{bp}") for bp in range(4)]
                    with tc.tile_pool(name="w2", bufs=2) as w2pool, \
                         tc.tile_pool(name="psc2", bufs=1, space="PSUM") as psc2pool:
                        p2ps = [psc2pool.tile([128, 256], F32, name="p2ps",
                                              tag=f"p2ps{bp}") for bp in range(4)]
                        nmm = [0, 0, 0, 0]
                        for g in range(9):
                            w2g = [w2pool.tile([128, 9, 256], F16, name="w2g",
                                               tag="w2g") for _ in range(2)]
                            for cic in range(2):
                                nc.sync.dma_start(out=w2g[cic],
                                                  in_=w2d[cic, :, g * 9:(g + 1) * 9, :])
                            for j in range(9):
                                khw = g * 9 + j
                                kh, kw = khw // 9, khw % 9
                                for cic in range(2):
                                    hshift = wpool.tile([128, B, 8, 8], F16,
                                                        name="hshift", tag="hshift")
                                    if cic == 0:
                                        nc.vector.tensor_copy(
                                            hshift,
                                            hsb[cic][:, :, kh:kh + 16:2, kw:kw + 16:2])
                                    else:
                                        nc.scalar.copy(
                                            hshift,
                                            hsb[cic][:, :, kh:kh + 16:2, kw:kw + 16:2])
                                    hflat = hshift.rearrange("p b h w -> p (b h w)")
                                    for bp in range(4):
                                        nc.tensor.matmul(
                                            p2ps[bp],
                                            lhsT=hflat[:, bp * 128:(bp + 1) * 128],
                                            rhs=w2g[cic][:, j, :],
                                            start=(nmm[bp] == 0), stop=(nmm[bp] == 161),
                                        )
                                        nmm[bp] += 1
                        for bp in range(4):
                            nc.vector.tensor_tensor(out=p2sb[bp], in0=p2ps[bp],
                                                    in1=pbrep, op=ALU.add)

                # stage C: squash over ow -> ub (bf16)
                ub = [apool.tile([128, 256], F16, name="ub", tag=f"ub{bp}")
                      for bp in range(4)]
                with tc.tile_pool(name="psn", bufs=2, space="PSUM") as psnpool:
                    for bp in range(4):
                        sq = wpool.tile([128, 256], F16, name="sq", tag="sq")
                        nc.vector.tensor_mul(sq, p2sb[bp], p2sb[bp])
                        n2ps = psnpool.tile([16, 256], F32, name="n2ps", tag="n2ps")
                        nc.tensor.matmul(n2ps, lhsT=gsb,
                                         rhs=sq, start=True, stop=True)
                        n2 = wpool.tile([16, 256], F32, name="n2", tag="n2")
                        nc.scalar.activation(n2, n2ps, AF.Copy)
                        r1 = wpool.tile([16, 256], F32, name="r1", tag="r1")
                        nc.vector.tensor_scalar_add(r1, in0=n2, scalar1=1.0)
                        nc.vector.reciprocal(r1, r1)
                        q = wpool.tile([16, 256], F32, name="q", tag="q")
                        nc.scalar.activation(q, n2, AF.Sqrt, bias=epssb[:16])
                        nc.vector.reciprocal(q, q)
                        f = wpool.tile([16, 256], F32, name="f", tag="f")
                        nc.vector.tensor_mul(f, n2, r1)
                        nc.vector.tensor_mul(f, f, q)
                        nc.sync.dma_start(out=fdram[bp], in_=f)
                        frep = wpool.tile([128, 256], F32, name="frep", tag="frep")
                        for grp in range(16):
                            nc.sync.dma_start(
                                out=frep[grp * 8:(grp + 1) * 8, :],
                                in_=bass.AP(tensor=fdram.tensor,
                                            offset=(bp * 16 + grp) * 256,
                                            ap=[[0, 8], [1, 256]]))
                        nc.vector.tensor_tensor(out=ub[bp], in0=p2sb[bp], in1=frep,
                                                op=ALU.mult)

                # stage D: u gathers -> ubig [128, cb, b]
                ubd = [wpool.tile([128, B, 64], F16, name="ubd", tag=f"ubd{H}")
                       for H in range(2)]
                for H in range(2):
                    for cp in range(2):
                        for b in range(B):
                            bp, bl = b // 2, b % 2
                            nc.sync.dma_start(
                                out=ubd[H][cp * 64:(cp + 1) * 64, b, :],
                                in_=ub[bp][bl * 64:(bl + 1) * 64,
                                           128 * H + 64 * cp:128 * H + 64 * cp + 64],
                            )
                for H in range(2):
                    nc.vector.tensor_copy(
                        ubig[:, 64 * H:64 * (H + 1), :],
                        bass.AP(tensor=ubd[H].tensor, offset=ubd[H].offset,
                                ap=[list(ubd[H].ap[0]), [1, 64], [64, B]]))

            # ---------- routing pools ----------
            vpool = stack.enter_context(tc.tile_pool(name="vsmall", bufs=1))
            wrpool = stack.enter_context(tc.tile_pool(name="wrp", bufs=3))
            uhppool = stack.enter_context(tc.tile_pool(name="uhp", bufs=2, space="PSUM"))
            psspool = stack.enter_context(tc.tile_pool(name="pss", bufs=1, space="PSUM"))
            ugpool = stack.enter_context(tc.tile_pool(name="ug", bufs=3))
            dmpool = stack.enter_context(tc.tile_pool(name="dmp", bufs=1))
            xpool = stack.enter_context(tc.tile_pool(name="xp", bufs=2))

            QS = (0, 512, 1024, 1536, 1600)

            def squash_dmaj(S, scale, final=False):
                """v = squash(S*scale); S psum [8, 1600] in d-major (d,o)."""
                S3 = S.rearrange("p (d o) -> p d o", d=16)
                sq = vpool.tile([8, 16, 100], F32, name="vsq", tag="vsq")
                nc.scalar.activation(sq.rearrange("p d o -> p (d o)"),
                                     S, AF.Square)
                nc.vector.tensor_tensor(out=sq[:, 0:8], in0=sq[:, 0:8],
                                        in1=sq[:, 8:16], op=ALU.add)
                nc.vector.tensor_tensor(out=sq[:, 0:4], in0=sq[:, 0:4],
                                        in1=sq[:, 4:8], op=ALU.add)
                nc.vector.tensor_tensor(out=sq[:, 0:2], in0=sq[:, 0:2],
                                        in1=sq[:, 2:4], op=ALU.add)
                n2 = vpool.tile([8, 100], F32, name="vn2", tag="vn2")
                nc.vector.tensor_tensor(out=n2, in0=sq[:, 0], in1=sq[:, 1],
                                        op=ALU.add)
                if scale != 1.0:
                    nc.vector.tensor_scalar_mul(n2, in0=n2, scalar1=scale * scale)
                r1 = vpool.tile([8, 100], F32, name="vr1", tag="vr1")
                nc.vector.tensor_scalar_add(r1, in0=n2, scalar1=1.0)
                nc.vector.reciprocal(r1, r1)
                q = vpool.tile([8, 100], F32, name="vq", tag="vq")
                nc.scalar.activation(q, n2, AF.Sqrt, bias=epssb[:8])
                nc.vector.reciprocal(q, q)
                f = vpool.tile([8, 100], F32, name="vf", tag="vf")
                nc.vector.tensor_mul(f, n2, r1)
                nc.vector.tensor_mul(f, f, q)
                if scale != 1.0:
                    nc.vector.tensor_scalar_mul(f, in0=f, scalar1=scale)
                if final:
                    nc.vector.tensor_tensor(
                        out=v2sb, in0=S3.transpose([0, 2, 1]),
                        in1=f.unsqueeze(2).broadcast_to([8, 100, 16]),
                        op=ALU.mult)
                    nc.sync.dma_start(out=vout, in_=v2sb)
                else:
                    v16 = vpool.tile([8, 16, 100], F16, name="v16", tag="v16")
                    nc.vector.tensor_tensor(
                        out=v16, in0=S3,
                        in1=f.unsqueeze(1).broadcast_to([8, 16, 100]),
                        op=ALU.mult)
                    nc.sync.dma_start(out=vdram,
                                      in_=v16.rearrange("p d o -> p (d o)"))
                    nc.sync.dma_start(
                        out=vrep.rearrange("p d o -> p (d o)"),
                        in_=bass.AP(tensor=vdram.tensor, offset=0,
                                    ap=[[0, 16], [1600, 8], [1, 1600]]))

            # ---------- pass 0: s0 = sum_i u_hat ----------
            s0ps = psspool.tile([8, 1600], F32, name="sps", tag="sps")
            for cb in range(128):
                wrt = wrpool.tile([128, 1600], F16, name="wrt", tag="wrt")
                eng = nc.sync if cb % 2 == 0 else nc.scalar
                eng.dma_start(out=wrt, in_=wrd[cb])
                for q in range(4):
                    n0, n1 = QS[q], QS[q + 1]
                    nc.tensor.matmul(s0ps[:, n0:n1],
                                     lhsT=ubig[:, cb, :],
                                     rhs=wrt[:, n0:n1],
                                     start=(cb == 0), stop=(cb == 127))
            squash_dmaj(s0ps, 0.01)

            # ---------- passes 1, 2 (software-pipelined) ----------
            NG = 128 // G

            GGD = slice(G // 2, G - 1)   # chunks 4..6 (DVE)
            GG7 = slice(G - 1, G)        # chunk 7 (GpSimd, contiguous)
            GD7 = slice(0, G - 1)        # chunks 0..6

            def head_half(uhg, dm, hh, gvb):
                """dm = uh*v and fold tree (in place), split DVE / GpSimd by
                whole chunks so every op reads contiguous rows."""
                if hh == 0:
                    gs = slice(0, G // 2)
                    nc.vector.tensor_tensor(out=dm[:, gs], in0=uhg[:, gs],
                                            in1=gvb[:, gs], op=ALU.mult)
                    nc.vector.tensor_tensor(out=dm[:, gs, 0:8], in0=dm[:, gs, 0:8],
                                            in1=dm[:, gs, 8:16], op=ALU.add)
                else:
                    nc.vector.tensor_tensor(out=dm[:, GGD], in0=uhg[:, GGD],
                                            in1=gvb[:, GGD], op=ALU.mult)
                    nc.gpsimd.tensor_tensor(out=dm[:, GG7], in0=uhg[:, GG7],
                                            in1=gvb[:, GG7], op=ALU.mult)
                    nc.vector.tensor_tensor(out=dm[:, GGD, 0:8],
                                            in0=dm[:, GGD, 0:8],
                                            in1=dm[:, GGD, 8:16], op=ALU.add)
                    nc.gpsimd.tensor_tensor(out=dm[:, GG7, 0:8],
                                            in0=dm[:, GG7, 0:8],
                                            in1=dm[:, GG7, 8:16], op=ALU.add)
                    nc.vector.tensor_tensor(out=dm[:, GD7, 0:4],
                                            in0=dm[:, GD7, 0:4],
                                            in1=dm[:, GD7, 4:8], op=ALU.add)
                    nc.gpsimd.tensor_tensor(out=dm[:, GG7, 0:4],
                                            in0=dm[:, GG7, 0:4],
                                            in1=dm[:, GG7, 4:8], op=ALU.add)
                    nc.gpsimd.tensor_tensor(out=dm[:, :, 0:2], in0=dm[:, :, 0:2],
                                            in1=dm[:, :, 2:4], op=ALU.add)

            for t in (1, 2):
                sps = psspool.tile([8, 1600], F32, name="sps", tag="sps")
                st = {}
                for gi in range(NG + 2):
                    # ---- softmax tail for group gi-1 ----
                    if 1 <= gi <= NG:
                        s = st[gi - 1]
                        e = xpool.tile([128, G, 100], F16, name="e", tag="e")
                        nc.scalar.activation(e.rearrange("p g o -> p (g o)"),
                                             s["logit"].rearrange("p g o -> p (g o)"),
                                             AF.Exp)
                        zf = xpool.tile([128, G], F32, name="zf", tag="zf")
                        nc.vector.tensor_reduce(zf, e, axis=AX.X, op=ALU.add)
                        zi = xpool.tile([128, G], F32, name="zi", tag="zi")
                        nc.vector.reciprocal(zi, zf)
                        zsel = xpool.tile([128, G, 8], F16, name="zsel", tag="zsel")
                        nc.vector.tensor_tensor(
                            out=zsel,
                            in0=sel16.unsqueeze(1).broadcast_to([128, G, 8]),
                            in1=zi.unsqueeze(2).broadcast_to([128, G, 8]),
                            op=ALU.mult)
                        s["zsel"] = zsel
                        uhg = s["uhg"]
                        eb = e.unsqueeze(2).broadcast_to([128, G, 16, 100])
                        nc.vector.tensor_tensor(out=uhg[:, GD7], in0=uhg[:, GD7],
                                                in1=eb[:, GD7], op=ALU.mult)
                        nc.gpsimd.tensor_tensor(out=uhg[:, GG7], in0=uhg[:, GG7],
                                                in1=eb[:, GG7], op=ALU.mult)
                    # ---- s-matmuls for group gi-2 ----
                    if 2 <= gi:
                        sp = st[gi - 2]
                        pf = sp["uhg"].rearrange("p g d o -> p (g d o)")
                        for j in range(G):
                            cb = sp["g0"] + j
                            for q in range(4):
                                n0, n1 = QS[q], QS[q + 1]
                                nc.tensor.matmul(
                                    sps[:, n0:n1], lhsT=sp["zsel"][:, j, :],
                                    rhs=pf[:, j * 1600 + n0:j * 1600 + n1],
                                    start=(cb == 0), stop=(cb == 127))
                        del st[gi - 2]
                    # ---- produce group gi + logit head ----
                    if gi < NG:
                        g0 = gi * G
                        # block-diag operand for this group
                        bdg = xpool.tile([128, G, 16, 8], F16, name="bdg", tag="bdg")
                        nc.vector.tensor_tensor(
                            out=bdg,
                            in0=ubig[:, g0:g0 + G].unsqueeze(2).broadcast_to(
                                [128, G, 16, 8]),
                            in1=masksb.unsqueeze(1).broadcast_to([128, G, 16, 8]),
                            op=ALU.mult)
                        uhg = ugpool.tile([128, G, 16, 100], F16, name="uhg",
                                          tag="uhg")
                        uhg_f = uhg.rearrange("p g d o -> p (g d o)")
                        dm = dmpool.tile([128, G, 16, 100], F16, name="dm", tag="dm")
                        gvb = vrep.unsqueeze(1).broadcast_to([128, G, 16, 100])
                        for j in range(G):
                            cb = g0 + j
                            wrt = wrpool.tile([128, 1600], F16, name="wrt", tag="wrt")
                            nc.sync.dma_start(out=wrt, in_=wrd[cb])
                            lhsT = bdg[:, j].rearrange("p i b -> p (i b)")
                            for h in range(2):
                                ps = uhppool.tile([128, 800], F32, name="uhps",
                                                  tag="uhps")
                                nc.tensor.matmul(ps[:, 0:512], lhsT=lhsT,
                                                 rhs=wrt[:, h * 800:h * 800 + 512],
                                                 start=True, stop=True)
                                nc.tensor.matmul(ps[:, 512:800], lhsT=lhsT,
                                                 rhs=wrt[:, h * 800 + 512:(h + 1) * 800],
                                                 start=True, stop=True)
                                nc.scalar.activation(
                                    uhg_f[:, j * 1600 + h * 800:j * 1600 + (h + 1) * 800],
                                    ps, AF.Copy)
                            if j == G // 2 - 1:
                                head_half(uhg, dm, 0, gvb)
                        head_half(uhg, dm, 1, gvb)
                        if t == 1:
                            logit = b1sb[:, g0:g0 + G]
                            nc.gpsimd.tensor_tensor(out=logit, in0=dm[:, :, 0],
                                                    in1=dm[:, :, 1], op=ALU.add)
                        else:
                            nc.gpsimd.tensor_tensor(out=dm[:, :, 0], in0=dm[:, :, 0],
                                                    in1=dm[:, :, 1], op=ALU.add)
                            logit = xpool.tile([128, G, 100], F16, name="lgt",
                                               tag="lgt")
                            nc.vector.tensor_tensor(out=logit, in0=dm[:, :, 0],
                                                    in1=b1sb[:, g0:g0 + G],
                                                    op=ALU.add)
                        st[gi] = {"g0": g0, "uhg": uhg, "logit": logit}
                squash_dmaj(sps, 1.0, final=(t == 2))

    nc.compile()
    return nc


def _host_prep(x, conv_w, conv_b, pcap_w, pcap_b, W):
    x = np.ascontiguousarray(np.asarray(x, np.float32))
    conv_w = np.asarray(conv_w, np.float32)
    conv_b = np.asarray(conv_b, np.float32)
    pcap_w = np.asarray(pcap_w, np.float32)
    pcap_b = np.asarray(pcap_b, np.float32)
    W = np.asarray(W, np.float32)

    # w1t[kh, (ci,kw), oc] = conv_w[oc, ci, kh, kw]
    w1t = np.ascontiguousarray(
        conv_w.transpose(2, 1, 3, 0).reshape(9, 27, 256)
    ).astype(np.float16)
    cb = np.ascontiguousarray(conv_b.reshape(256, 1))
    w2t = np.ascontiguousarray(
        pcap_w.transpose(1, 2, 3, 0).reshape(2, 128, 81, 256)
    ).astype(np.float16)
    pb = np.ascontiguousarray(pcap_b.reshape(1, 256))
    # wr[cb=(H,r)][p=(cp,oh,ow)][(d,o)] = W[o, (128H+64cp+r)*8+oh, d, ow]
    arr = W.transpose(1, 3, 0, 2)                # [i=2048, k=8, o=100, d=16]
    arr = arr.reshape(2, 2, 64, 8, 8, 100, 16)   # [H, cp, r, oh, k, o, d]
    arr = arr.transpose(0, 2, 1, 3, 4, 6, 5)     # [H, r, cp, oh, k, d, o]
    wr = np.ascontiguousarray(arr.reshape(128, 128, 1600)).astype(np.float16)

    mask = np.zeros((128, 16, 8), np.float32)
    for p in range(128):
        mask[p, p // 8, :] = 1.0
    mask = mask.astype(np.float16)
    sel = np.zeros((128, 8), np.float32)
    for p in range(128):
        sel[p, p % 8] = 1.0
    sel = sel.astype(np.float16)
    g = np.zeros((128, 16), np.float32)
    for p in range(128):
        g[p, p // 8] = 1.0
    g = g.astype(np.float16)

    shared = {"w1t": w1t, "cb": cb, "w2t": w2t, "pb": pb, "wr": wr,
              "mask": mask, "sel": sel, "gmat": g}
    in_maps = []
    for c in range(N_CORES):
        m = dict(shared)
        m["x_sh"] = np.ascontiguousarray(x[c * B:(c + 1) * B])
        in_maps.append(m)
    return in_maps


def run(inputs, trace=False, **kw):
    key = "nc"
    if key not in _CACHE:
        _CACHE[key] = _build()
    nc = _CACHE[key]
    in_maps = _host_prep(**inputs)
    res = bass_utils.run_bass_kernel_spmd(
        nc, in_maps, core_ids=list(range(N_CORES)), trace=trace, **kw)
    return res


def kernel(**inputs):
    res = run(inputs)
    v = np.concatenate([res.results[i]["v_out"] for i in range(N_CORES)], axis=0)
    return v


# revision 29
# speedup vs baseline: 1.0483x; 1.0483x over previous
"""CapsNet-CIFAR100 forward pass on 8 Trainium2 NeuronCores.

Data-parallel over batch (8 images/core). Conv stem + primary caps as
matmuls (f32r / bf16 fast-path streaming); dynamic routing with every
26M-element u_hat pass produced and consumed at bf16:

  pass 0: s0 = sum_i u_hat via dense-u matmuls (u_hat never formed),
          wr streamed bf16.
  pass 1/2: software-pipelined groups of G=8 chunks.  Per iteration gi:
      exp(gi-1) [ACT] -> softmax tail of gi-1 (z, 1/z, zsel, p16) [DVE+Pool]
      -> i-sum s-matmuls of gi-2 [PE] -> produce group gi (wr DMA, block-diag
      bf16 matmuls -> PSUM halves, ACT exits to bf16 SBUF, d-major [(d,o)])
      -> b-logit head of gi (dm = uh*v, fold tree in place) [DVE+Pool].
  The 1/z softmax normalizer is folded into the i-sum matmul's selector
  operand (zsel), so c never materializes.

Capsule chunking: chunk cb in 0..127, H=cb//64, r=cb%64; the chunk's 16
capsules are (cp in {0,1}, oh in 0..7) with ch = 128H+64cp+r, i = ch*8+oh,
vector dim k = ow. Partition index within chunk: p = cp*64 + oh*8 + ow.
conv2 runs "transposed" (output partitions = (b%2, oh, ow), free = co) so
the u gather is 32 contiguous [64,64] SBUF DMAs.
"""

from contextlib import ExitStack

import numpy as np
import ml_dtypes
import concourse.bass as bass
import concourse.mybir as mybir
import concourse.tile as tile
from concourse import bacc
from concourse import bass_utils

F32 = mybir.dt.float32
F32R = mybir.dt.float32r
F16 = mybir.dt.float16
AF = mybir.ActivationFunctionType
ALU = mybir.AluOpType
AX = mybir.AxisListType

N_CORES = 8
B = 8            # batch per core
G = 8            # routing chunks per consumer group
OSPL = 84        # o-split: [0:OSPL] on DVE, [OSPL:100] on GpSimd
EPS = 1e-8

_CACHE = {}


def _build():
    nc = bacc.Bacc("TRN2", target_bir_lowering=False, debug=False,
                   num_devices=N_CORES)

    xd = nc.dram_tensor("x_sh", [B, 3, 32, 32], F32, kind="ExternalInput").ap()
    w1d = nc.dram_tensor("w1t", [9, 27, 256], F16, kind="ExternalInput").ap()
    cbd = nc.dram_tensor("cb", [256, 1], F32, kind="ExternalInput").ap()
    w2d = nc.dram_tensor("w2t", [2, 128, 81, 256], F16, kind="ExternalInput").ap()
    pbd = nc.dram_tensor("pb", [1, 256], F32, kind="ExternalInput").ap()
    wrd = nc.dram_tensor("wr", [128, 128, 1600], F16, kind="ExternalInput").ap()
    mkd = nc.dram_tensor("mask", [128, 16, 8], F16, kind="ExternalInput").ap()
    seld = nc.dram_tensor("sel", [128, 8], F16, kind="ExternalInput").ap()
    gd = nc.dram_tensor("gmat", [128, 16], F16, kind="ExternalInput").ap()
    fdram = nc.dram_tensor("fscratch", [4, 16, 256], F32, kind="Internal").ap()
    vdram = nc.dram_tensor("vscratch", [8, 1600], F16, kind="Internal").ap()
    vout = nc.dram_tensor("v_out", [B, 100, 16], F32, kind="ExternalOutput").ap()

    with tile.TileContext(nc) as tc:
        with ExitStack() as stack:
            cpool = stack.enter_context(tc.tile_pool(name="consts", bufs=1))
            rpool = stack.enter_context(tc.tile_pool(name="rconsts", bufs=1))

            # ---------- shared constants ----------
            w1sb = cpool.tile([27, 9, 256], F16, name="w1sb")
            nc.sync.dma_start(out=w1sb, in_=w1d.rearrange("k c o -> c k o"))
            cbsb = cpool.tile([128, 2, 1], F32, name="cbsb")
            nc.sync.dma_start(out=cbsb, in_=cbd.rearrange("(t p) one -> p t one", p=128))
            pbrep = cpool.tile([128, 256], F32, name="pbrep")
            nc.sync.dma_start(
                out=pbrep,
                in_=bass.AP(tensor=pbd.tensor, offset=0, ap=[[0, 128], [1, 256]]))
            epssb = cpool.tile([128, 1], F32, name="epssb")
            nc.vector.memset(epssb, EPS)
            gsb = cpool.tile([128, 16], F16, name="gsb")
            nc.sync.dma_start(out=gsb, in_=gd)
            masksb = cpool.tile([128, 16, 8], F16, name="masksb")
            nc.sync.dma_start(out=masksb, in_=mkd)
            sel16 = cpool.tile([128, 8], F16, name="sel16")
            nc.sync.dma_start(out=sel16, in_=seld)

            # routing-persistent tiles
            ubig = rpool.tile([128, 128, B], F16, name="ubig")
            vrep = rpool.tile([128, 16, 100], F16, name="vrep")
            b1sb = rpool.tile([128, 128, 100], F16, name="b1sb")
            v2sb = rpool.tile([8, 100, 16], F32, name="v2sb")

            # ---------- conv stages (scoped pools; freed before routing) ----
            with tc.tile_pool(name="work", bufs=2) as wpool, \
                 tc.tile_pool(name="acts", bufs=1) as apool:
                # stage A: conv1 [B,3,32,32] -> h [256, B, 24, 24]
                # xsb[(ci,kw), b, r, w] = x[b, ci, r, w+kw]; contraction over
                # (ci,kw)=27, accumulate over kh in PSUM.
                with tc.tile_pool(name="hpool", bufs=1) as hpool:
                    hsb = [hpool.tile([128, B, 24, 24], F16, name="hsb",
                                      tag=f"h{c}") for c in range(2)]
                    with tc.tile_pool(name="imp", bufs=1) as impool, \
                         tc.tile_pool(name="psc", bufs=2, space="PSUM") as pscpool:
                        xsf = impool.tile([27, B, 32, 24], F32, name="xsf")
                        for ci in range(3):
                            for kw in range(9):
                                src = bass.AP(
                                    tensor=xd.tensor,
                                    offset=ci * 1024 + kw,
                                    ap=[[3072, B], [32, 32], [1, 24]],
                                )
                                nc.sync.dma_start(
                                    out=xsf[ci * 9 + kw:ci * 9 + kw + 1], in_=src)
                        xsb = impool.tile([27, B, 32, 24], F16, name="xsb")
                        nc.vector.tensor_copy(xsb, xsf)

                        for oc in range(2):
                            for b in range(B):
                                for hh in range(2):
                                    ph = pscpool.tile([128, 288], F32, name="ph",
                                                      tag="pconv")
                                    for kh in range(9):
                                        nc.tensor.matmul(
                                            ph,
                                            lhsT=w1sb[:, kh, oc * 128:(oc + 1) * 128],
                                            rhs=xsb[:, b, kh + hh * 12:
                                                    kh + hh * 12 + 12, :].rearrange(
                                                        "c h w -> c (h w)"),
                                            start=(kh == 0), stop=(kh == 8),
                                        )
                                    nc.scalar.activation(
                                        hsb[oc][:, b, hh * 12:(hh + 1) * 12, :].rearrange(
                                            "p h w -> p (h w)"),
                                        ph, AF.Relu, bias=cbsb[:, oc],
                                    )

                    # stage B: conv2 (transposed) -> p2sb
                    p2sb = [apool.tile([128, 256], F32, name="p2sb",
                                       tag=f"p2sb{bp}") for bp in range(4)]
                    with tc.tile_pool(name="w2", bufs=2) as w2pool, \
                         tc.tile_pool(name="psc2", bufs=1, space="PSUM") as psc2pool:
                        p2ps = [psc2pool.tile([128, 256], F32, name="p2ps",
                                              tag=f"p2ps{bp}") for bp in range(4)]
                        nmm = [0, 0, 0, 0]
                        for g in range(9):
                            w2g = [w2pool.tile([128, 9, 256], F16, name="w2g",
                                               tag="w2g") for _ in range(2)]
                            for cic in range(2):
                                nc.sync.dma_start(out=w2g[cic],
                                                  in_=w2d[cic, :, g * 9:(g + 1) * 9, :])
                            for j in range(9):
                                khw = g * 9 + j
                                kh, kw = khw // 9, khw % 9
                                for cic in range(2):
                                    hshift = wpool.tile([128, B, 8, 8], F16,
                                                        name="hshift", tag="hshift")
                                    if cic == 0:
                                        nc.vector.tensor_copy(
                                            hshift,
                                            hsb[cic][:, :, kh:kh + 16:2, kw:kw + 16:2])
                                    else:
                                        nc.scalar.copy(
                                            hshift,
                                            hsb[cic][:, :, kh:kh + 16:2, kw:kw + 16:2])
                                    hflat = hshift.rearrange("p b h w -> p (b h w)")
                                    for bp in range(4):
                                        nc.tensor.matmul(
                                            p2ps[bp],
                                            lhsT=hflat[:, bp * 128:(bp + 1) * 128],
                                            rhs=w2g[cic][:, j, :],
                                            start=(nmm[bp] == 0), stop=(nmm[bp] == 161),
                                        )
                                        nmm[bp] += 1
                        for bp in range(4):
                            nc.vector.tensor_tensor(out=p2sb[bp], in0=p2ps[bp],
                                                    in1=pbrep, op=ALU.add)

                # stage C: squash over ow -> ub (bf16)
                ub = [apool.tile([128, 256], F16, name="ub", tag=f"ub{bp}")
                      for bp in range(4)]
                with tc.tile_pool(name="psn", bufs=2, space="PSUM") as psnpool:
                    for bp in range(4):
                        sq = wpool.tile([128, 256], F16, name="sq", tag="sq")
                        nc.vector.tensor_mul(sq, p2sb[bp], p2sb[bp])
                        n2ps = psnpool.tile([16, 256], F32, name="n2ps", tag="n2ps")
                        nc.tensor.matmul(n2ps, lhsT=gsb,
                                         rhs=sq, start=True, stop=True)
                        n2 = wpool.tile([16, 256], F32, name="n2", tag="n2")
                        nc.scalar.activation(n2, n2ps, AF.Copy)
                        r1 = wpool.tile([16, 256], F32, name="r1", tag="r1")
                        nc.vector.tensor_scalar_add(r1, in0=n2, scalar1=1.0)
                        nc.vector.reciprocal(r1, r1)
                        q = wpool.tile([16, 256], F32, name="q", tag="q")
                        nc.scalar.activation(q, n2, AF.Sqrt, bias=epssb[:16])
                        nc.vector.reciprocal(q, q)
                        f = wpool.tile([16, 256], F32, name="f", tag="f")
                        nc.vector.tensor_mul(f, n2, r1)
                        nc.vector.tensor_mul(f, f, q)
                        nc.sync.dma_start(out=fdram[bp], in_=f)
                        frep = wpool.tile([128, 256], F32, name="frep", tag="frep")
                        for grp in range(16):
                            nc.sync.dma_start(
                                out=frep[grp * 8:(grp + 1) * 8, :],
                                in_=bass.AP(tensor=fdram.tensor,
                                            offset=(bp * 16 + grp) * 256,
                                            ap=[[0, 8], [1, 256]]))
                        nc.vector.tensor_tensor(out=ub[bp], in0=p2sb[bp], in1=frep,
                                                op=ALU.mult)

                # stage D: u gathers -> ubig [128, cb, b]
                ubd = [wpool.tile([128, B, 64], F16, name="ubd", tag=f"ubd{H}")
                       for H in range(2)]
                for H in range(2):
                    for cp in range(2):
                        for b in range(B):
                            bp, bl = b // 2, b % 2
                            nc.sync.dma_start(
                                out=ubd[H][cp * 64:(cp + 1) * 64, b, :],
                                in_=ub[bp][bl * 64:(bl + 1) * 64,
                                           128 * H + 64 * cp:128 * H + 64 * cp + 64],
                            )
                for H in range(2):
                    nc.vector.tensor_copy(
                        ubig[:, 64 * H:64 * (H + 1), :],
                        bass.AP(tensor=ubd[H].tensor, offset=ubd[H].offset,
                                ap=[list(ubd[H].ap[0]), [1, 64], [64, B]]))

            # ---------- routing pools ----------
            vpool = stack.enter_context(tc.tile_pool(name="vsmall", bufs=1))
            wrpool = stack.enter_context(tc.tile_pool(name="wrp", bufs=6))
            uhppool = stack.enter_context(tc.tile_pool(name="uhp", bufs=2, space="PSUM"))
            psspool = stack.enter_context(tc.tile_pool(name="pss", bufs=1, space="PSUM"))
            ugpool = stack.enter_context(tc.tile_pool(name="ug", bufs=3))
            dmpool = stack.enter_context(tc.tile_pool(name="dmp", bufs=1))
            xpool = stack.enter_context(tc.tile_pool(name="xp", bufs=2))

            QS = (0, 512, 1024, 1536, 1600)

            def squash_dmaj(S, scale, final=False):
                """v = squash(S*scale); S psum [8, 1600] in d-major (d,o)."""
                S3 = S.rearrange("p (d o) -> p d o", d=16)
                sq = vpool.tile([8, 16, 100], F32, name="vsq", tag="vsq")
                nc.scalar.activation(sq.rearrange("p d o -> p (d o)"),
                                     S, AF.Square)
                nc.vector.tensor_tensor(out=sq[:, 0:8], in0=sq[:, 0:8],
                                        in1=sq[:, 8:16], op=ALU.add)
                nc.vector.tensor_tensor(out=sq[:, 0:4], in0=sq[:, 0:4],
                                        in1=sq[:, 4:8], op=ALU.add)
                nc.vector.tensor_tensor(out=sq[:, 0:2], in0=sq[:, 0:2],
                                        in1=sq[:, 2:4], op=ALU.add)
                n2 = vpool.tile([8, 100], F32, name="vn2", tag="vn2")
                nc.vector.tensor_tensor(out=n2, in0=sq[:, 0], in1=sq[:, 1],
                                        op=ALU.add)
                if scale != 1.0:
                    nc.vector.tensor_scalar_mul(n2, in0=n2, scalar1=scale * scale)
                r1 = vpool.tile([8, 100], F32, name="vr1", tag="vr1")
                nc.vector.tensor_scalar_add(r1, in0=n2, scalar1=1.0)
                nc.vector.reciprocal(r1, r1)
                q = vpool.tile([8, 100], F32, name="vq", tag="vq")
                nc.scalar.activation(q, n2, AF.Sqrt, bias=epssb[:8])
                nc.vector.reciprocal(q, q)
                f = vpool.tile([8, 100], F32, name="vf", tag="vf")
                nc.vector.tensor_mul(f, n2, r1)
                nc.vector.tensor_mul(f, f, q)
                if scale != 1.0:
                    nc.vector.tensor_scalar_mul(f, in0=f, scalar1=scale)
                if final:
                    nc.vector.tensor_tensor(
                        out=v2sb, in0=S3.transpose([0, 2, 1]),
                        in1=f.unsqueeze(2).broadcast_to([8, 100, 16]),
                        op=ALU.mult)
                    nc.sync.dma_start(out=vout, in_=v2sb)
                else:
                    v16 = vpool.tile([8, 16, 100], F16, name="v16", tag="v16")
                    nc.vector.tensor_tensor(
                        out=v16, in0=S3,
                        in1=f.unsqueeze(1).broadcast_to([8, 16, 100]),
                        op=ALU.mult)
                    nc.sync.dma_start(out=vdram,
                                      in_=v16.rearrange("p d o -> p (d o)"))
                    nc.sync.dma_start(
                        out=vrep.rearrange("p d o -> p (d o)"),
                        in_=bass.AP(tensor=vdram.tensor, offset=0,
                                    ap=[[0, 16], [1600, 8], [1, 1600]]))

            # ---------- pass 0: s0 = sum_i u_hat ----------
            s0ps = psspool.tile([8, 1600], F32, name="sps", tag="sps")
            for cb in range(128):
                wrt = wrpool.tile([128, 1600], F16, name="wrt", tag="wrt")
                eng = nc.sync if cb % 2 == 0 else nc.scalar
                eng.dma_start(out=wrt, in_=wrd[cb])
                for q in range(4):
                    n0, n1 = QS[q], QS[q + 1]
                    nc.tensor.matmul(s0ps[:, n0:n1],
                                     lhsT=ubig[:, cb, :],
                                     rhs=wrt[:, n0:n1],
                                     start=(cb == 0), stop=(cb == 127))
            squash_dmaj(s0ps, 0.01)

            # ---------- passes 1, 2 (software-pipelined) ----------
            NG = 128 // G

            GGD = slice(G // 2, G - 1)   # chunks 4..6 (DVE)
            GG7 = slice(G - 1, G)        # chunk 7 (GpSimd, contiguous)
            GD7 = slice(0, G - 1)        # chunks 0..6

            def head_half(uhg, dm, hh, gvb):
                """dm = uh*v and fold tree (in place), split DVE / GpSimd by
                whole chunks so every op reads contiguous rows."""
                if hh == 0:
                    gs = slice(0, G // 2)
                    nc.vector.tensor_tensor(out=dm[:, gs], in0=uhg[:, gs],
                                            in1=gvb[:, gs], op=ALU.mult)
                    nc.vector.tensor_tensor(out=dm[:, gs, 0:8], in0=dm[:, gs, 0:8],
                                            in1=dm[:, gs, 8:16], op=ALU.add)
                else:
                    nc.vector.tensor_tensor(out=dm[:, GGD], in0=uhg[:, GGD],
                                            in1=gvb[:, GGD], op=ALU.mult)
                    nc.gpsimd.tensor_tensor(out=dm[:, GG7], in0=uhg[:, GG7],
                                            in1=gvb[:, GG7], op=ALU.mult)
                    nc.vector.tensor_tensor(out=dm[:, GGD, 0:8],
                                            in0=dm[:, GGD, 0:8],
                                            in1=dm[:, GGD, 8:16], op=ALU.add)
                    nc.gpsimd.tensor_tensor(out=dm[:, GG7, 0:8],
                                            in0=dm[:, GG7, 0:8],
                                            in1=dm[:, GG7, 8:16], op=ALU.add)
                    nc.vector.tensor_tensor(out=dm[:, GD7, 0:4],
                                            in0=dm[:, GD7, 0:4],
                                            in1=dm[:, GD7, 4:8], op=ALU.add)
                    nc.gpsimd.tensor_tensor(out=dm[:, GG7, 0:4],
                                            in0=dm[:, GG7, 0:4],
                                            in1=dm[:, GG7, 4:8], op=ALU.add)
                    nc.gpsimd.tensor_tensor(out=dm[:, :, 0:2], in0=dm[:, :, 0:2],
                                            in1=dm[:, :, 2:4], op=ALU.add)

            for t in (1, 2):
                sps = psspool.tile([8, 1600], F32, name="sps", tag="sps")
                st = {}
                for gi in range(NG + 2):
                    # ---- softmax tail for group gi-1 ----
                    if 1 <= gi <= NG:
                        s = st[gi - 1]
                        e = xpool.tile([128, G, 100], F16, name="e", tag="e")
                        nc.scalar.activation(e.rearrange("p g o -> p (g o)"),
                                             s["logit"].rearrange("p g o -> p (g o)"),
                                             AF.Exp)
                        zf = xpool.tile([128, G], F32, name="zf", tag="zf")
                        nc.vector.tensor_reduce(zf, e, axis=AX.X, op=ALU.add)
                        zi = xpool.tile([128, G], F32, name="zi", tag="zi")
                        nc.vector.reciprocal(zi, zf)
                        zsel = xpool.tile([128, G, 8], F16, name="zsel", tag="zsel")
                        nc.vector.tensor_tensor(
                            out=zsel,
                            in0=sel16.unsqueeze(1).broadcast_to([128, G, 8]),
                            in1=zi.unsqueeze(2).broadcast_to([128, G, 8]),
                            op=ALU.mult)
                        s["zsel"] = zsel
                        uhg = s["uhg"]
                        eb = e.unsqueeze(2).broadcast_to([128, G, 16, 100])
                        nc.vector.tensor_tensor(out=uhg[:, GD7], in0=uhg[:, GD7],
                                                in1=eb[:, GD7], op=ALU.mult)
                        nc.gpsimd.tensor_tensor(out=uhg[:, GG7], in0=uhg[:, GG7],
                                                in1=eb[:, GG7], op=ALU.mult)
                    # ---- s-matmuls for group gi-2 ----
                    if 2 <= gi:
                        sp = st[gi - 2]
                        pf = sp["uhg"].rearrange("p g d o -> p (g d o)")
                        for j in range(G):
                            cb = sp["g0"] + j
                            for q in range(4):
                                n0, n1 = QS[q], QS[q + 1]
                                nc.tensor.matmul(
                                    sps[:, n0:n1], lhsT=sp["zsel"][:, j, :],
                                    rhs=pf[:, j * 1600 + n0:j * 1600 + n1],
                                    start=(cb == 0), stop=(cb == 127))
                        del st[gi - 2]
                    # ---- produce group gi + logit head ----
                    if gi < NG:
                        g0 = gi * G
                        # block-diag operand for this group
                        bdg = xpool.tile([128, G, 16, 8], F16, name="bdg", tag="bdg")
                        nc.vector.tensor_tensor(
                            out=bdg,
                            in0=ubig[:, g0:g0 + G].unsqueeze(2).broadcast_to(
                                [128, G, 16, 8]),
                            in1=masksb.unsqueeze(1).broadcast_to([128, G, 16, 8]),
                            op=ALU.mult)
                        uhg = ugpool.tile([128, G, 16, 100], F16, name="uhg",
                                          tag="uhg")
                        uhg_f = uhg.rearrange("p g d o -> p (g d o)")
                        dm = dmpool.tile([128, G, 16, 100], F16, name="dm", tag="dm")
                        gvb = vrep.unsqueeze(1).broadcast_to([128, G, 16, 100])
                        for j in range(G):
                            cb = g0 + j
                            wrt = wrpool.tile([128, 1600], F16, name="wrt", tag="wrt")
                            nc.sync.dma_start(out=wrt, in_=wrd[cb])
                            lhsT = bdg[:, j].rearrange("p i b -> p (i b)")
                            for h in range(2):
                                ps = uhppool.tile([128, 800], F32, name="uhps",
                                                  tag="uhps")
                                nc.tensor.matmul(ps[:, 0:512], lhsT=lhsT,
                                                 rhs=wrt[:, h * 800:h * 800 + 512],
                                                 start=True, stop=True)
                                nc.tensor.matmul(ps[:, 512:800], lhsT=lhsT,
                                                 rhs=wrt[:, h * 800 + 512:(h + 1) * 800],
                                                 start=True, stop=True)
                                nc.scalar.activation(
                                    uhg_f[:, j * 1600 + h * 800:j * 1600 + (h + 1) * 800],
                                    ps, AF.Copy)
                            if j == G // 2 - 1:
                                head_half(uhg, dm, 0, gvb)
                        head_half(uhg, dm, 1, gvb)
                        if t == 1:
                            logit = b1sb[:, g0:g0 + G]
                            nc.gpsimd.tensor_tensor(out=logit, in0=dm[:, :, 0],
                                                    in1=dm[:, :, 1], op=ALU.add)
                        else:
                            nc.gpsimd.tensor_tensor(out=dm[:, :, 0], in0=dm[:, :, 0],
                                                    in1=dm[:, :, 1], op=ALU.add)
                            logit = xpool.tile([128, G, 100], F16, name="lgt",
                                               tag="lgt")
                            nc.vector.tensor_tensor(out=logit, in0=dm[:, :, 0],
                                                    in1=b1sb[:, g0:g0 + G],
                                                    op=ALU.add)
                        st[gi] = {"g0": g0, "uhg": uhg, "logit": logit}
                squash_dmaj(sps, 1.0, final=(t == 2))

    nc.compile()
    return nc


def _host_prep(x, conv_w, conv_b, pcap_w, pcap_b, W):
    x = np.ascontiguousarray(np.asarray(x, np.float32))
    conv_w = np.asarray(conv_w, np.float32)
    conv_b = np.asarray(conv_b, np.float32)
    pcap_w = np.asarray(pcap_w, np.float32)
    pcap_b = np.asarray(pcap_b, np.float32)
    W = np.asarray(W, np.float32)

    # w1t[kh, (ci,kw), oc] = conv_w[oc, ci, kh, kw]
    w1t = np.ascontiguousarray(
        conv_w.transpose(2, 1, 3, 0).reshape(9, 27, 256)
    ).astype(np.float16)
    cb = np.ascontiguousarray(conv_b.reshape(256, 1))
    w2t = np.ascontiguousarray(
        pcap_w.transpose(1, 2, 3, 0).reshape(2, 128, 81, 256)
    ).astype(np.float16)
    pb = np.ascontiguousarray(pcap_b.reshape(1, 256))
    # wr[cb=(H,r)][p=(cp,oh,ow)][(d,o)] = W[o, (128H+64cp+r)*8+oh, d, ow]
    arr = W.transpose(1, 3, 0, 2)                # [i=2048, k=8, o=100, d=16]
    arr = arr.reshape(2, 2, 64, 8, 8, 100, 16)   # [H, cp, r, oh, k, o, d]
    arr = arr.transpose(0, 2, 1, 3, 4, 6, 5)     # [H, r, cp, oh, k, d, o]
    wr = np.ascontiguousarray(arr.reshape(128, 128, 1600)).astype(np.float16)

    mask = np.zeros((128, 16, 8), np.float32)
    for p in range(128):
        mask[p, p // 8, :] = 1.0
    mask = mask.astype(np.float16)
    sel = np.zeros((128, 8), np.float32)
    for p in range(128):
        sel[p, p % 8] = 1.0
    sel = sel.astype(np.float16)
    g = np.zeros((128, 16), np.float32)
    for p in range(128):
        g[p, p // 8] = 1.0
    g = g.astype(np.float16)

    shared = {"w1t": w1t, "cb": cb, "w2t": w2t, "pb": pb, "wr": wr,
              "mask": mask, "sel": sel, "gmat": g}
    in_maps = []
    for c in range(N_CORES):
        m = dict(shared)
        m["x_sh"] = np.ascontiguousarray(x[c * B:(c + 1) * B])
        in_maps.append(m)
    return in_maps


def run(inputs, trace=False, **kw):
    key = "nc"
    if key not in _CACHE:
        _CACHE[key] = _build()
    nc = _CACHE[key]
    in_maps = _host_prep(**inputs)
    res = bass_utils.run_bass_kernel_spmd(
        nc, in_maps, core_ids=list(range(N_CORES)), trace=trace, **kw)
    return res


def kernel(**inputs):
    res = run(inputs)
    v = np.concatenate([res.results[i]["v_out"] for i in range(N_CORES)], axis=0)
    return v


# revision 30
# speedup vs baseline: 1.0664x; 1.0173x over previous
"""CapsNet-CIFAR100 forward pass on 8 Trainium2 NeuronCores.

Data-parallel over batch (8 images/core). Conv stem + primary caps as
matmuls (f32r / bf16 fast-path streaming); dynamic routing with every
26M-element u_hat pass produced and consumed at bf16:

  pass 0: s0 = sum_i u_hat via dense-u matmuls (u_hat never formed),
          wr streamed bf16.
  pass 1/2: software-pipelined groups of G=8 chunks.  Per iteration gi:
      exp(gi-1) [ACT] -> softmax tail of gi-1 (z, 1/z, zsel, p16) [DVE+Pool]
      -> i-sum s-matmuls of gi-2 [PE] -> produce group gi (wr DMA, block-diag
      bf16 matmuls -> PSUM halves, ACT exits to bf16 SBUF, d-major [(d,o)])
      -> b-logit head of gi (dm = uh*v, fold tree in place) [DVE+Pool].
  The 1/z softmax normalizer is folded into the i-sum matmul's selector
  operand (zsel), so c never materializes.

Capsule chunking: chunk cb in 0..127, H=cb//64, r=cb%64; the chunk's 16
capsules are (cp in {0,1}, oh in 0..7) with ch = 128H+64cp+r, i = ch*8+oh,
vector dim k = ow. Partition index within chunk: p = cp*64 + oh*8 + ow.
conv2 runs "transposed" (output partitions = (b%2, oh, ow), free = co) so
the u gather is 32 contiguous [64,64] SBUF DMAs.
"""

from contextlib import ExitStack

import numpy as np
import ml_dtypes
import concourse.bass as bass
import concourse.mybir as mybir
import concourse.tile as tile
from concourse import bacc
from concourse import bass_utils

F32 = mybir.dt.float32
F32R = mybir.dt.float32r
F16 = mybir.dt.float16
AF = mybir.ActivationFunctionType
ALU = mybir.AluOpType
AX = mybir.AxisListType

N_CORES = 8
B = 8            # batch per core
G = 8            # routing chunks per consumer group
OSPL = 84        # o-split: [0:OSPL] on DVE, [OSPL:100] on GpSimd
EPS = 1e-8

_CACHE = {}


def _build():
    nc = bacc.Bacc("TRN2", target_bir_lowering=False, debug=False,
                   num_devices=N_CORES)

    xd = nc.dram_tensor("x_sh", [B, 3, 32, 32], F32, kind="ExternalInput").ap()
    w1d = nc.dram_tensor("w1t", [9, 27, 256], F16, kind="ExternalInput").ap()
    cbd = nc.dram_tensor("cb", [256, 1], F32, kind="ExternalInput").ap()
    w2d = nc.dram_tensor("w2t", [2, 128, 81, 256], F16, kind="ExternalInput").ap()
    pbd = nc.dram_tensor("pb", [1, 256], F32, kind="ExternalInput").ap()
    wrd = nc.dram_tensor("wr", [128, 128, 1600], F16, kind="ExternalInput").ap()
    mkd = nc.dram_tensor("mask", [128, 16, 8], F16, kind="ExternalInput").ap()
    seld = nc.dram_tensor("sel", [128, 8], F16, kind="ExternalInput").ap()
    gd = nc.dram_tensor("gmat", [128, 16], F16, kind="ExternalInput").ap()
    fdram = nc.dram_tensor("fscratch", [4, 16, 256], F32, kind="Internal").ap()
    vdram = nc.dram_tensor("vscratch", [8, 1600], F16, kind="Internal").ap()
    vout = nc.dram_tensor("v_out", [B, 100, 16], F32, kind="ExternalOutput").ap()

    with tile.TileContext(nc) as tc:
        with ExitStack() as stack:
            cpool = stack.enter_context(tc.tile_pool(name="consts", bufs=1))
            rpool = stack.enter_context(tc.tile_pool(name="rconsts", bufs=1))

            # ---------- shared constants ----------
            w1sb = cpool.tile([27, 9, 256], F16, name="w1sb")
            nc.sync.dma_start(out=w1sb, in_=w1d.rearrange("k c o -> c k o"))
            cbsb = cpool.tile([128, 2, 1], F32, name="cbsb")
            nc.sync.dma_start(out=cbsb, in_=cbd.rearrange("(t p) one -> p t one", p=128))
            pbrep = cpool.tile([128, 256], F32, name="pbrep")
            nc.sync.dma_start(
                out=pbrep,
                in_=bass.AP(tensor=pbd.tensor, offset=0, ap=[[0, 128], [1, 256]]))
            epssb = cpool.tile([128, 1], F32, name="epssb")
            nc.vector.memset(epssb, EPS)
            gsb = cpool.tile([128, 16], F16, name="gsb")
            nc.sync.dma_start(out=gsb, in_=gd)
            masksb = cpool.tile([128, 16, 8], F16, name="masksb")
            nc.sync.dma_start(out=masksb, in_=mkd)
            sel16 = cpool.tile([128, 8], F16, name="sel16")
            nc.sync.dma_start(out=sel16, in_=seld)

            # routing-persistent tiles
            ubig = rpool.tile([128, 128, B], F16, name="ubig")
            vrep = rpool.tile([128, 16, 100], F16, name="vrep")
            b1sb = rpool.tile([128, 128, 100], F16, name="b1sb")
            v2sb = rpool.tile([8, 100, 16], F32, name="v2sb")

            # ---------- conv stages (scoped pools; freed before routing) ----
            with tc.tile_pool(name="work", bufs=2) as wpool, \
                 tc.tile_pool(name="acts", bufs=1) as apool:
                # stage A: conv1 [B,3,32,32] -> h [256, B, 24, 24]
                # xsb[(ci,kw), b, r, w] = x[b, ci, r, w+kw]; contraction over
                # (ci,kw)=27, accumulate over kh in PSUM.
                with tc.tile_pool(name="hpool", bufs=1) as hpool:
                    hsb = [hpool.tile([128, B, 24, 24], F16, name="hsb",
                                      tag=f"h{c}") for c in range(2)]
                    with tc.tile_pool(name="imp", bufs=1) as impool, \
                         tc.tile_pool(name="psc", bufs=2, space="PSUM") as pscpool:
                        xsf = impool.tile([27, B, 32, 24], F32, name="xsf")
                        for ci in range(3):
                            for kw in range(9):
                                src = bass.AP(
                                    tensor=xd.tensor,
                                    offset=ci * 1024 + kw,
                                    ap=[[3072, B], [32, 32], [1, 24]],
                                )
                                nc.sync.dma_start(
                                    out=xsf[ci * 9 + kw:ci * 9 + kw + 1], in_=src)
                        xsb = impool.tile([27, B, 32, 24], F16, name="xsb")
                        nc.vector.tensor_copy(xsb, xsf)

                        for oc in range(2):
                            for b in range(B):
                                for hh in range(2):
                                    ph = pscpool.tile([128, 288], F32, name="ph",
                                                      tag="pconv")
                                    for kh in range(9):
                                        nc.tensor.matmul(
                                            ph,
                                            lhsT=w1sb[:, kh, oc * 128:(oc + 1) * 128],
                                            rhs=xsb[:, b, kh + hh * 12:
                                                    kh + hh * 12 + 12, :].rearrange(
                                                        "c h w -> c (h w)"),
                                            start=(kh == 0), stop=(kh == 8),
                                        )
                                    nc.scalar.activation(
                                        hsb[oc][:, b, hh * 12:(hh + 1) * 12, :].rearrange(
                                            "p h w -> p (h w)"),
                                        ph, AF.Relu, bias=cbsb[:, oc],
                                    )

                    # stage B: conv2 (transposed) -> p2sb
                    p2sb = [apool.tile([128, 256], F32, name="p2sb",
                                       tag=f"p2sb{bp}") for bp in range(4)]
                    with tc.tile_pool(name="w2", bufs=2) as w2pool, \
                         tc.tile_pool(name="psc2", bufs=1, space="PSUM") as psc2pool:
                        p2ps = [psc2pool.tile([128, 256], F32, name="p2ps",
                                              tag=f"p2ps{bp}") for bp in range(4)]
                        nmm = [0, 0, 0, 0]
                        for g in range(9):
                            w2g = [w2pool.tile([128, 9, 256], F16, name="w2g",
                                               tag="w2g") for _ in range(2)]
                            for cic in range(2):
                                nc.sync.dma_start(out=w2g[cic],
                                                  in_=w2d[cic, :, g * 9:(g + 1) * 9, :])
                            for j in range(9):
                                khw = g * 9 + j
                                kh, kw = khw // 9, khw % 9
                                for cic in range(2):
                                    hshift = wpool.tile([128, B, 8, 8], F16,
                                                        name="hshift", tag="hshift")
                                    if cic == 0:
                                        nc.vector.tensor_copy(
                                            hshift,
                                            hsb[cic][:, :, kh:kh + 16:2, kw:kw + 16:2])
                                    else:
                                        nc.scalar.copy(
                                            hshift,
                                            hsb[cic][:, :, kh:kh + 16:2, kw:kw + 16:2])
                                    hflat = hshift.rearrange("p b h w -> p (b h w)")
                                    for bp in range(4):
                                        nc.tensor.matmul(
                                            p2ps[bp],
                                            lhsT=hflat[:, bp * 128:(bp + 1) * 128],
                                            rhs=w2g[cic][:, j, :],
                                            start=(nmm[bp] == 0), stop=(nmm[bp] == 161),
                                        )
                                        nmm[bp] += 1
                        for bp in range(4):
                            nc.vector.tensor_tensor(out=p2sb[bp], in0=p2ps[bp],
                                                    in1=pbrep, op=ALU.add)

                # stage C: squash over ow -> ub (bf16)
                ub = [apool.tile([128, 256], F16, name="ub", tag=f"ub{bp}")
                      for bp in range(4)]
                with tc.tile_pool(name="psn", bufs=2, space="PSUM") as psnpool:
                    for bp in range(4):
                        sq = wpool.tile([128, 256], F16, name="sq", tag="sq")
                        nc.vector.tensor_mul(sq, p2sb[bp], p2sb[bp])
                        n2ps = psnpool.tile([16, 256], F32, name="n2ps", tag="n2ps")
                        nc.tensor.matmul(n2ps, lhsT=gsb,
                                         rhs=sq, start=True, stop=True)
                        n2 = wpool.tile([16, 256], F32, name="n2", tag="n2")
                        nc.scalar.activation(n2, n2ps, AF.Copy)
                        r1 = wpool.tile([16, 256], F32, name="r1", tag="r1")
                        nc.vector.tensor_scalar_add(r1, in0=n2, scalar1=1.0)
                        nc.vector.reciprocal(r1, r1)
                        q = wpool.tile([16, 256], F32, name="q", tag="q")
                        nc.scalar.activation(q, n2, AF.Sqrt, bias=epssb[:16])
                        nc.vector.reciprocal(q, q)
                        f = wpool.tile([16, 256], F32, name="f", tag="f")
                        nc.vector.tensor_mul(f, n2, r1)
                        nc.vector.tensor_mul(f, f, q)
                        nc.sync.dma_start(out=fdram[bp], in_=f)
                        frep = wpool.tile([128, 256], F32, name="frep", tag="frep")
                        for grp in range(16):
                            nc.sync.dma_start(
                                out=frep[grp * 8:(grp + 1) * 8, :],
                                in_=bass.AP(tensor=fdram.tensor,
                                            offset=(bp * 16 + grp) * 256,
                                            ap=[[0, 8], [1, 256]]))
                        nc.vector.tensor_tensor(out=ub[bp], in0=p2sb[bp], in1=frep,
                                                op=ALU.mult)

                # stage D: u gathers -> ubig [128, cb, b]
                ubd = [wpool.tile([128, B, 64], F16, name="ubd", tag=f"ubd{H}")
                       for H in range(2)]
                for H in range(2):
                    for cp in range(2):
                        for b in range(B):
                            bp, bl = b // 2, b % 2
                            nc.sync.dma_start(
                                out=ubd[H][cp * 64:(cp + 1) * 64, b, :],
                                in_=ub[bp][bl * 64:(bl + 1) * 64,
                                           128 * H + 64 * cp:128 * H + 64 * cp + 64],
                            )
                for H in range(2):
                    nc.vector.tensor_copy(
                        ubig[:, 64 * H:64 * (H + 1), :],
                        bass.AP(tensor=ubd[H].tensor, offset=ubd[H].offset,
                                ap=[list(ubd[H].ap[0]), [1, 64], [64, B]]))

            # ---------- routing pools ----------
            vpool = stack.enter_context(tc.tile_pool(name="vsmall", bufs=1))
            wrpool = stack.enter_context(tc.tile_pool(name="wrp", bufs=10))
            uhppool = stack.enter_context(tc.tile_pool(name="uhp", bufs=2, space="PSUM"))
            psspool = stack.enter_context(tc.tile_pool(name="pss", bufs=1, space="PSUM"))
            ugpool = stack.enter_context(tc.tile_pool(name="ug", bufs=3))
            dmpool = stack.enter_context(tc.tile_pool(name="dmp", bufs=1))
            xpool = stack.enter_context(tc.tile_pool(name="xp", bufs=2))

            QS = (0, 512, 1024, 1536, 1600)

            def squash_dmaj(S, scale, final=False):
                """v = squash(S*scale); S psum [8, 1600] in d-major (d,o)."""
                S3 = S.rearrange("p (d o) -> p d o", d=16)
                sq = vpool.tile([8, 16, 100], F32, name="vsq", tag="vsq")
                nc.scalar.activation(sq.rearrange("p d o -> p (d o)"),
                                     S, AF.Square)
                nc.vector.tensor_tensor(out=sq[:, 0:8], in0=sq[:, 0:8],
                                        in1=sq[:, 8:16], op=ALU.add)
                nc.vector.tensor_tensor(out=sq[:, 0:4], in0=sq[:, 0:4],
                                        in1=sq[:, 4:8], op=ALU.add)
                nc.vector.tensor_tensor(out=sq[:, 0:2], in0=sq[:, 0:2],
                                        in1=sq[:, 2:4], op=ALU.add)
                n2 = vpool.tile([8, 100], F32, name="vn2", tag="vn2")
                nc.vector.tensor_tensor(out=n2, in0=sq[:, 0], in1=sq[:, 1],
                                        op=ALU.add)
                if scale != 1.0:
                    nc.vector.tensor_scalar_mul(n2, in0=n2, scalar1=scale * scale)
                r1 = vpool.tile([8, 100], F32, name="vr1", tag="vr1")
                nc.vector.tensor_scalar_add(r1, in0=n2, scalar1=1.0)
                nc.vector.reciprocal(r1, r1)
                q = vpool.tile([8, 100], F32, name="vq", tag="vq")
                nc.scalar.activation(q, n2, AF.Sqrt, bias=epssb[:8])
                nc.vector.reciprocal(q, q)
                f = vpool.tile([8, 100], F32, name="vf", tag="vf")
                nc.vector.tensor_mul(f, n2, r1)
                nc.vector.tensor_mul(f, f, q)
                if scale != 1.0:
                    nc.vector.tensor_scalar_mul(f, in0=f, scalar1=scale)
                if final:
                    nc.vector.tensor_tensor(
                        out=v2sb, in0=S3.transpose([0, 2, 1]),
                        in1=f.unsqueeze(2).broadcast_to([8, 100, 16]),
                        op=ALU.mult)
                    nc.sync.dma_start(out=vout, in_=v2sb)
                else:
                    v16 = vpool.tile([8, 16, 100], F16, name="v16", tag="v16")
                    nc.vector.tensor_tensor(
                        out=v16, in0=S3,
                        in1=f.unsqueeze(1).broadcast_to([8, 16, 100]),
                        op=ALU.mult)
                    nc.sync.dma_start(out=vdram,
                                      in_=v16.rearrange("p d o -> p (d o)"))
                    nc.sync.dma_start(
                        out=vrep.rearrange("p d o -> p (d o)"),
                        in_=bass.AP(tensor=vdram.tensor, offset=0,
                                    ap=[[0, 16], [1600, 8], [1, 1600]]))

            # ---------- pass 0: s0 = sum_i u_hat ----------
            s0ps = psspool.tile([8, 1600], F32, name="sps", tag="sps")
            for cb in range(128):
                wrt = wrpool.tile([128, 1600], F16, name="wrt", tag="wrt")
                eng = nc.sync if cb % 2 == 0 else nc.scalar
                eng.dma_start(out=wrt, in_=wrd[cb])
                for q in range(4):
                    n0, n1 = QS[q], QS[q + 1]
                    nc.tensor.matmul(s0ps[:, n0:n1],
                                     lhsT=ubig[:, cb, :],
                                     rhs=wrt[:, n0:n1],
                                     start=(cb == 0), stop=(cb == 127))
            squash_dmaj(s0ps, 0.01)

            # ---------- passes 1, 2 (software-pipelined) ----------
            NG = 128 // G

            GGD = slice(G // 2, G - 1)   # chunks 4..6 (DVE)
            GG7 = slice(G - 1, G)        # chunk 7 (GpSimd, contiguous)
            GD7 = slice(0, G - 1)        # chunks 0..6

            def head_half(uhg, dm, hh, gvb):
                """dm = uh*v and fold tree (in place), split DVE / GpSimd by
                whole chunks so every op reads contiguous rows."""
                if hh == 0:
                    gs = slice(0, G // 2)
                    nc.vector.tensor_tensor(out=dm[:, gs], in0=uhg[:, gs],
                                            in1=gvb[:, gs], op=ALU.mult)
                    nc.vector.tensor_tensor(out=dm[:, gs, 0:8], in0=dm[:, gs, 0:8],
                                            in1=dm[:, gs, 8:16], op=ALU.add)
                else:
                    nc.vector.tensor_tensor(out=dm[:, GGD], in0=uhg[:, GGD],
                                            in1=gvb[:, GGD], op=ALU.mult)
                    nc.gpsimd.tensor_tensor(out=dm[:, GG7], in0=uhg[:, GG7],
                                            in1=gvb[:, GG7], op=ALU.mult)
                    nc.vector.tensor_tensor(out=dm[:, GGD, 0:8],
                                            in0=dm[:, GGD, 0:8],
                                            in1=dm[:, GGD, 8:16], op=ALU.add)
                    nc.gpsimd.tensor_tensor(out=dm[:, GG7, 0:8],
                                            in0=dm[:, GG7, 0:8],
                                            in1=dm[:, GG7, 8:16], op=ALU.add)
                    nc.vector.tensor_tensor(out=dm[:, GD7, 0:4],
                                            in0=dm[:, GD7, 0:4],
                                            in1=dm[:, GD7, 4:8], op=ALU.add)
                    nc.gpsimd.tensor_tensor(out=dm[:, GG7, 0:4],
                                            in0=dm[:, GG7, 0:4],
                                            in1=dm[:, GG7, 4:8], op=ALU.add)
                    nc.gpsimd.tensor_tensor(out=dm[:, :, 0:2], in0=dm[:, :, 0:2],
                                            in1=dm[:, :, 2:4], op=ALU.add)

            for t in (1, 2):
                sps = psspool.tile([8, 1600], F32, name="sps", tag="sps")
                st = {}
                for gi in range(NG + 2):
                    # ---- softmax tail for group gi-1 ----
                    if 1 <= gi <= NG:
                        s = st[gi - 1]
                        e = xpool.tile([128, G, 100], F16, name="e", tag="e")
                        nc.scalar.activation(e.rearrange("p g o -> p (g o)"),
                                             s["logit"].rearrange("p g o -> p (g o)"),
                                             AF.Exp)
                        zf = xpool.tile([128, G], F32, name="zf", tag="zf")
                        nc.vector.tensor_reduce(zf, e, axis=AX.X, op=ALU.add)
                        zi = xpool.tile([128, G], F32, name="zi", tag="zi")
                        nc.vector.reciprocal(zi, zf)
                        zsel = xpool.tile([128, G, 8], F16, name="zsel", tag="zsel")
                        nc.vector.tensor_tensor(
                            out=zsel,
                            in0=sel16.unsqueeze(1).broadcast_to([128, G, 8]),
                            in1=zi.unsqueeze(2).broadcast_to([128, G, 8]),
                            op=ALU.mult)
                        s["zsel"] = zsel
                        uhg = s["uhg"]
                        eb = e.unsqueeze(2).broadcast_to([128, G, 16, 100])
                        nc.vector.tensor_tensor(out=uhg[:, GD7], in0=uhg[:, GD7],
                                                in1=eb[:, GD7], op=ALU.mult)
                        nc.gpsimd.tensor_tensor(out=uhg[:, GG7], in0=uhg[:, GG7],
                                                in1=eb[:, GG7], op=ALU.mult)
                    # ---- s-matmuls for group gi-2 ----
                    if 2 <= gi:
                        sp = st[gi - 2]
                        pf = sp["uhg"].rearrange("p g d o -> p (g d o)")
                        for j in range(G):
                            cb = sp["g0"] + j
                            for q in range(4):
                                n0, n1 = QS[q], QS[q + 1]
                                nc.tensor.matmul(
                                    sps[:, n0:n1], lhsT=sp["zsel"][:, j, :],
                                    rhs=pf[:, j * 1600 + n0:j * 1600 + n1],
                                    start=(cb == 0), stop=(cb == 127))
                        del st[gi - 2]
                    # ---- produce group gi + logit head ----
                    if gi < NG:
                        g0 = gi * G
                        # block-diag operand for this group
                        bdg = xpool.tile([128, G, 16, 8], F16, name="bdg", tag="bdg")
                        nc.vector.tensor_tensor(
                            out=bdg,
                            in0=ubig[:, g0:g0 + G].unsqueeze(2).broadcast_to(
                                [128, G, 16, 8]),
                            in1=masksb.unsqueeze(1).broadcast_to([128, G, 16, 8]),
                            op=ALU.mult)
                        uhg = ugpool.tile([128, G, 16, 100], F16, name="uhg",
                                          tag="uhg")
                        uhg_f = uhg.rearrange("p g d o -> p (g d o)")
                        dm = dmpool.tile([128, G, 16, 100], F16, name="dm", tag="dm")
                        gvb = vrep.unsqueeze(1).broadcast_to([128, G, 16, 100])
                        for j in range(G):
                            cb = g0 + j
                            wrt = wrpool.tile([128, 1600], F16, name="wrt", tag="wrt")
                            nc.sync.dma_start(out=wrt, in_=wrd[cb])
                            lhsT = bdg[:, j].rearrange("p i b -> p (i b)")
                            for h in range(2):
                                ps = uhppool.tile([128, 800], F32, name="uhps",
                                                  tag="uhps")
                                nc.tensor.matmul(ps[:, 0:512], lhsT=lhsT,
                                                 rhs=wrt[:, h * 800:h * 800 + 512],
                                                 start=True, stop=True)
                                nc.tensor.matmul(ps[:, 512:800], lhsT=lhsT,
                                                 rhs=wrt[:, h * 800 + 512:(h + 1) * 800],
                                                 start=True, stop=True)
                                nc.scalar.activation(
                                    uhg_f[:, j * 1600 + h * 800:j * 1600 + (h + 1) * 800],
                                    ps, AF.Copy)
                            if j == G // 2 - 1:
                                head_half(uhg, dm, 0, gvb)
                        head_half(uhg, dm, 1, gvb)
                        if t == 1:
                            logit = b1sb[:, g0:g0 + G]
                            nc.gpsimd.tensor_tensor(out=logit, in0=dm[:, :, 0],
                                                    in1=dm[:, :, 1], op=ALU.add)
                        else:
                            nc.gpsimd.tensor_tensor(out=dm[:, :, 0], in0=dm[:, :, 0],
                                                    in1=dm[:, :, 1], op=ALU.add)
                            logit = xpool.tile([128, G, 100], F16, name="lgt",
                                               tag="lgt")
                            nc.vector.tensor_tensor(out=logit, in0=dm[:, :, 0],
                                                    in1=b1sb[:, g0:g0 + G],
                                                    op=ALU.add)
                        st[gi] = {"g0": g0, "uhg": uhg, "logit": logit}
                squash_dmaj(sps, 1.0, final=(t == 2))

    nc.compile()
    return nc


def _host_prep(x, conv_w, conv_b, pcap_w, pcap_b, W):
    x = np.ascontiguousarray(np.asarray(x, np.float32))
    conv_w = np.asarray(conv_w, np.float32)
    conv_b = np.asarray(conv_b, np.float32)
    pcap_w = np.asarray(pcap_w, np.float32)
    pcap_b = np.asarray(pcap_b, np.float32)
    W = np.asarray(W, np.float32)

    # w1t[kh, (ci,kw), oc] = conv_w[oc, ci, kh, kw]
    w1t = np.ascontiguousarray(
        conv_w.transpose(2, 1, 3, 0).reshape(9, 27, 256)
    ).astype(np.float16)
    cb = np.ascontiguousarray(conv_b.reshape(256, 1))
    w2t = np.ascontiguousarray(
        pcap_w.transpose(1, 2, 3, 0).reshape(2, 128, 81, 256)
    ).astype(np.float16)
    pb = np.ascontiguousarray(pcap_b.reshape(1, 256))
    # wr[cb=(H,r)][p=(cp,oh,ow)][(d,o)] = W[o, (128H+64cp+r)*8+oh, d, ow]
    arr = W.transpose(1, 3, 0, 2)                # [i=2048, k=8, o=100, d=16]
    arr = arr.reshape(2, 2, 64, 8, 8, 100, 16)   # [H, cp, r, oh, k, o, d]
    arr = arr.transpose(0, 2, 1, 3, 4, 6, 5)     # [H, r, cp, oh, k, d, o]
    wr = np.ascontiguousarray(arr.reshape(128, 128, 1600)).astype(np.float16)

    mask = np.zeros((128, 16, 8), np.float32)
    for p in range(128):
        mask[p, p // 8, :] = 1.0
    mask = mask.astype(np.float16)
    sel = np.zeros((128, 8), np.float32)
    for p in range(128):
        sel[p, p % 8] = 1.0
    sel = sel.astype(np.float16)
    g = np.zeros((128, 16), np.float32)
    for p in range(128):
        g[p, p // 8] = 1.0
    g = g.astype(np.float16)

    shared = {"w1t": w1t, "cb": cb, "w2t": w2t, "pb": pb, "wr": wr,
              "mask": mask, "sel": sel, "gmat": g}
    in_maps = []
    for c in range(N_CORES):
        m = dict(shared)
        m["x_sh"] = np.ascontiguousarray(x[c * B:(c + 1) * B])
        in_maps.append(m)
    return in_maps


def run(inputs, trace=False, **kw):
    key = "nc"
    if key not in _CACHE:
        _CACHE[key] = _build()
    nc = _CACHE[key]
    in_maps = _host_prep(**inputs)
    res = bass_utils.run_bass_kernel_spmd(
        nc, in_maps, core_ids=list(range(N_CORES)), trace=trace, **kw)
    return res


def kernel(**inputs):
    res = run(inputs)
    v = np.concatenate([res.results[i]["v_out"] for i in range(N_CORES)], axis=0)
    return v
